# revision 14
# baseline (speedup 1.0000x reference)
"""Trainium2 Bass kernel for batched single-head attention with seq-sum pooling.

Reference computation (B=16, S=2048, D=512, fp32):
    q = x @ W_q ; k = x @ W_k ; v = x @ W_v          per batch  [S, D]
    scores = q @ k.T / sqrt(D)                        [S, S]
    attn = softmax(scores, axis=-1)
    out_b = sum_s (attn @ v)[s, :]                    [D]

Key algebraic restructure: the final sum over query positions commutes with
the attn @ v matmul:
    out_b = (sum_q attn[q, :]) @ v = (r^T E) @ v
where E = exp(scores / sqrt(D)) and r[q] = 1 / rowsum_q(E).  This removes the
second [S,S]x[S,D] matmul (~36% of the FLOPs) and replaces it with one
[1,S]x[S,S] column-sum matmul plus one [1,S]x[S,D] matvec.

Sharding: pure data parallelism over batch — 2 batch elements per core on 8
NeuronCores, weights replicated, no collectives.  Host concatenates per-core
[2, D] outputs.

Matmuls run as float32r (fp32 data, relaxed-precision PE mode, 4x fp32
throughput; measured rel error ~1.5e-4).  Emission is software-pipelined so
the PE sees a dense matmul stream: X-transposes run one s-chunk ahead woven
between projection groups, batch 0's w-phase weaves into batch 1's
projections, and batch 1's V projections are deferred to weave through the
tail w-phase — PE-idle windows >3.4us would drop the HAM clock gate from
2.4GHz to 1.2GHz (transpose-mode matmuls do not count as PE activity).
"""

import sys

sys.path.insert(0, "/opt/trn_rl_repo")

import numpy as np

import concourse.bass as bass
import concourse.mybir as mybir
import concourse.tile as tile
from concourse import bacc
from concourse.bass_utils import run_bass_kernel_spmd
from concourse.masks import make_identity

B, S, D = 16, 2048, 512
P = 128
N_CORES = 8
B_PER_CORE = B // N_CORES  # 2
SCALE = 1.0 / float(np.sqrt(D))

F32 = mybir.dt.float32
F32R = mybir.dt.float32r

N_ST = S // P  # 16 s-tiles (partition tiles of the sequence dim)
N_DT = D // P  # 4 d-tiles (partition tiles of the feature dim)
NCH = 512  # moving free dim per matmul (one PSUM bank of fp32)
N_SC = S // NCH  # 4 s-chunks of the sequence dim
N_KC = S // NCH  # 4 k-chunks of the key dim


def build_nc():
    nc = bacc.Bacc("TRN2", target_bir_lowering=False, debug=False, num_devices=N_CORES)
    x_ext = nc.dram_tensor(
        "inputs", [B_PER_CORE, S, D], F32, kind="ExternalInput"
    ).ap()
    wq_ext = nc.dram_tensor("W_q", [D, D], F32, kind="ExternalInput").ap()
    wk_ext = nc.dram_tensor("W_k", [D, D], F32, kind="ExternalInput").ap()
    wv_ext = nc.dram_tensor("W_v", [D, D], F32, kind="ExternalInput").ap()
    out_ext = nc.dram_tensor("out", [B_PER_CORE, D], F32, kind="ExternalOutput").ap()

    with tile.TileContext(nc) as tc:
        with (
            tc.tile_pool(name="const", bufs=1) as const_pool,
            tc.tile_pool(name="w", bufs=1) as w_pool,
            tc.tile_pool(name="xin", bufs=2) as xin_pool,
            tc.tile_pool(name="xt", bufs=1) as xt_pool,
            tc.tile_pool(name="qkv", bufs=1) as qkv_pool,
            tc.tile_pool(name="e", bufs=2) as e_pool,
            tc.tile_pool(name="soft", bufs=4) as soft_pool,
            tc.tile_pool(name="wvec", bufs=1) as wvec_pool,
            tc.tile_pool(name="scps", bufs=2, space="PSUM") as sc_psum,
            tc.tile_pool(name="gpps", bufs=2, space="PSUM") as gp_psum,
            tc.tile_pool(name="wps", bufs=1, space="PSUM") as w_psum,
        ):
            ident_f = const_pool.tile([P, P], F32)
            make_identity(nc, ident_f[:])
            ident = const_pool.tile([P, P], F32R)
            nc.vector.tensor_copy(ident[:], ident_f[:])
            one_f = const_pool.tile([1, 2], F32)
            nc.gpsimd.memset(one_f[:], 1.0)
            one_t = const_pool.tile([1, 2], F32R)
            nc.vector.tensor_copy(one_t[:], one_f[:])

            # First x chunk of batch 0 before the weight DMAs — it gates the
            # first PE transposes, all share the SWDGE (cast) queue.
            x_first = xin_pool.tile([P, 4, D], F32R, tag="xin")
            nc.gpsimd.dma_start(
                out=x_first[:],
                in_=x_ext[0, 0:NCH, :].rearrange("(t p) d -> p t d", p=P),
            )

            # Weights: [D, D] -> per-contraction-tile layout [P, N_DT, D], f32r
            w_tiles = []
            for name, ext in (("wk", wk_ext), ("wq", wq_ext), ("wv", wv_ext)):
                w_s = w_pool.tile([P, N_DT, D], F32R, tag=name)
                nc.gpsimd.dma_start(
                    out=w_s[:], in_=ext.rearrange("(t p) e -> p t e", p=P)
                )
                w_tiles.append(w_s)
            wk_s, wq_s, wv_s = w_tiles

            # ---------- thunk builders (emission deferred for interleaving) --

            def ltp_thunks(b, x_first_tile=None):
                """Load + transpose + QT/KT projection thunks for batch b.

                Returns (handles, dma_thunks[sc], trans_thunks[sc] (16 each),
                kq_group_thunks[sc] (8 each), v_group_thunks (16))."""
                xt_s = xt_pool.tile([P, N_DT, S], F32R, tag="xt")
                qt_s = qkv_pool.tile([P, N_DT, S], F32R, tag="qt")
                kt_s = qkv_pool.tile([P, N_DT, S], F32R, tag="kt")
                v_s = qkv_pool.tile([P, N_ST, D], F32R, tag="v")
                x_tiles = [None] * N_SC
                if x_first_tile is not None:
                    x_tiles[0] = x_first_tile

                def make_dma(sc):
                    def th():
                        x_tile = xin_pool.tile([P, 4, D], F32R, tag="xin")
                        nc.gpsimd.dma_start(
                            out=x_tile[:],
                            in_=x_ext[
                                b, sc * NCH : (sc + 1) * NCH, :
                            ].rearrange("(t p) d -> p t d", p=P),
                        )
                        x_tiles[sc] = x_tile

                    return th

                def make_trans(sc, t_i, dt_i):
                    def th():
                        st = sc * 4 + t_i
                        tp = gp_psum.tile([P, P], F32R, tag="gp")
                        nc.tensor.transpose(
                            tp[:],
                            x_tiles[sc][:, t_i, dt_i * P : (dt_i + 1) * P],
                            ident[:],
                        )
                        nc.vector.tensor_copy(
                            xt_s[:, dt_i, st * P : (st + 1) * P], tp[:]
                        )

                    return th

                def make_kq(sc, w_src, dst, et):
                    def th():
                        mp = gp_psum.tile([P, NCH], F32, tag="gp")
                        for kd in range(N_DT):
                            nc.tensor.matmul(
                                mp[:],
                                w_src[:, kd, et * P : (et + 1) * P],
                                xt_s[:, kd, sc * NCH : (sc + 1) * NCH],
                                start=(kd == 0),
                                stop=(kd == N_DT - 1),
                            )
                        nc.vector.tensor_copy(
                            dst[:, et, sc * NCH : (sc + 1) * NCH], mp[:]
                        )

                    return th

                def make_v(st):
                    def th():
                        mp = gp_psum.tile([P, NCH], F32, tag="gp")
                        for kd in range(N_DT):
                            nc.tensor.matmul(
                                mp[:],
                                xt_s[:, kd, st * P : (st + 1) * P],
                                wv_s[:, kd, :],
                                start=(kd == 0),
                                stop=(kd == N_DT - 1),
                            )
                        nc.vector.tensor_copy(v_s[:, st, :], mp[:])

                    return th

                dma_th = [None if sc == 0 and x_tiles[0] is not None else make_dma(sc)
                          for sc in range(N_SC)]
                trans_th = [
                    [make_trans(sc, t_i, dt_i) for t_i in range(4) for dt_i in range(N_DT)]
                    for sc in range(N_SC)
                ]
                kq_th = [
                    [make_kq(sc, w_src, dst, et)
                     for w_src, dst in ((wk_s, kt_s), (wq_s, qt_s))
                     for et in range(N_DT)]
                    for sc in range(N_SC)
                ]
                v_th = [make_v(st) for st in range(N_ST)]
                return (qt_s, kt_s, v_s), dma_th, trans_th, kq_th, v_th

            def emit_ltp(dma_th, trans_th, kq_th, extra=None):
                """Emit load/transpose/KQ-projection stream.  Transposes for
                chunk sc+1 and `extra` thunks weave between chunk sc's
                projection groups so the PE stream stays dense."""
                extra = list(extra) if extra else []
                ei = 0
                if dma_th[0] is not None:
                    dma_th[0]()
                for th in trans_th[0]:
                    th()
                for sc in range(N_SC):
                    if sc + 1 < N_SC and dma_th[sc + 1] is not None:
                        dma_th[sc + 1]()
                    nxt = trans_th[sc + 1] if sc + 1 < N_SC else []
                    groups = kq_th[sc]
                    per = (len(nxt) + len(groups) - 1) // len(groups)
                    ti = 0
                    for g_i, g in enumerate(groups):
                        g()
                        for _ in range(per):
                            if ti < len(nxt):
                                nxt[ti]()
                                ti += 1
                        if ei < len(extra):
                            extra[ei]()
                            ei += 1
                    while ti < len(nxt):
                        nxt[ti]()
                        ti += 1
                while ei < len(extra):
                    extra[ei]()
                    ei += 1

            def emit_scores_qt(qt_s, kt_s, qt):
                """scores + exp + rowsum + reciprocal for one q-tile."""
                e_t = e_pool.tile([P, S], F32R, tag="e")
                rsum = soft_pool.tile([P, N_KC], F32, tag="rsum")
                for kc in range(N_KC):
                    sp = sc_psum.tile([P, NCH], F32, tag="sc")
                    for et in range(N_DT):
                        nc.tensor.matmul(
                            sp[:],
                            qt_s[:, et, qt * P : (qt + 1) * P],
                            kt_s[:, et, kc * NCH : (kc + 1) * NCH],
                            start=(et == 0),
                            stop=(et == N_DT - 1),
                        )
                    nc.scalar.activation(
                        e_t[:, kc * NCH : (kc + 1) * NCH],
                        sp[:],
                        mybir.ActivationFunctionType.Exp,
                        scale=SCALE,
                        accum_out=rsum[:, kc : kc + 1],
                    )
                rtot = soft_pool.tile([P, 1], F32, tag="rtot")
                nc.vector.reduce_sum(rtot[:], rsum[:], axis=mybir.AxisListType.X)
                rrec = soft_pool.tile([P, 1], F32, tag="rrec")
                nc.vector.reciprocal(rrec[:], rtot[:])
                # f32r matmuls need a full 128-wide stationary operand; broadcast
                # r across all columns so every PSUM output row equals r^T E.
                r_t = soft_pool.tile([P, P], F32R, tag="r")
                nc.vector.tensor_copy(r_t[:], rrec[:, 0:1].broadcast_to([P, P]))
                return e_t, r_t

            def emit_colsum_qt(w_ps, e_t, r_t, qt):
                """w_ps[:, kc, :] += bcast(r_qt)^T @ E_qt (every row = colsum)."""
                for kc in range(N_KC):
                    nc.tensor.matmul(
                        w_ps[:, kc, :],
                        r_t[:],
                        e_t[:, kc * NCH : (kc + 1) * NCH],
                        start=(qt == 0),
                        stop=(qt == N_ST - 1),
                        skip_group_check=True,
                    )

            def phase_scores(b, qt_s, kt_s):
                w_ps = w_psum.tile([P, N_KC, NCH], F32, tag="w")
                prev = None
                for qt in range(N_ST):
                    cur = emit_scores_qt(qt_s, kt_s, qt)
                    if prev is not None:
                        emit_colsum_qt(w_ps, prev[0], prev[1], qt - 1)
                    prev = cur
                emit_colsum_qt(w_ps, prev[0], prev[1], N_ST - 1)
                return w_ps

            def final_thunks(b, w_ps, v_s):
                """w-phase thunks: 4 ACT copies of w, then 16 (PE transpose +
                DVE broadcast), then 16 final accumulation matmuls, then the
                output copy + DMA.  Emitted interleaved by the caller."""
                w_sb = wvec_pool.tile([1, S], F32R, tag="wsb")
                o_ps = sc_psum.tile([P, NCH], F32, tag="sc")
                wt_pads = {}
                thunks = []

                def make_wcopy(kc):
                    def th():
                        nc.scalar.copy(
                            w_sb[:, kc * NCH : (kc + 1) * NCH], w_ps[0:1, kc, :]
                        )

                    return th

                def make_wtrans(kt):
                    def th():
                        tp = gp_psum.tile([P, 2], F32, tag="gp")
                        nc.tensor.matmul(
                            tp[:],
                            w_sb[0:1, kt * P : (kt + 1) * P],
                            one_t[0:1, 0:2],
                            start=True,
                            stop=True,
                        )
                        wt_pad = wvec_pool.tile([P, P], F32R, tag=f"wtp{kt % 4}")
                        nc.vector.tensor_copy(
                            wt_pad[:], tp[:, 0:1].broadcast_to([P, P])
                        )
                        wt_pads[kt] = wt_pad

                    return th

                def make_final(st):
                    def th():
                        nc.tensor.matmul(
                            o_ps[:],
                            wt_pads[st][:],
                            v_s[:, st, :],
                            start=(st == 0),
                            stop=(st == N_ST - 1),
                            skip_group_check=True,
                        )

                    return th

                def make_out():
                    def th():
                        o_sb = wvec_pool.tile([1, NCH], F32, tag="osb")
                        nc.scalar.copy(o_sb[:], o_ps[0:1, :])
                        nc.sync.dma_start(out=out_ext[b : b + 1, :], in_=o_sb[:])

                    return th

                for kc in range(N_KC):
                    thunks.append(make_wcopy(kc))
                for kt in range(N_ST):
                    thunks.append(make_wtrans(kt))
                    if kt >= 3:
                        thunks.append(make_final(kt - 3))
                for st in range(N_ST - 3, N_ST):
                    thunks.append(make_final(st))
                thunks.append(make_out())
                return thunks

            # ------------------------- emission ------------------------------

            # batch 0: transposes woven into KQ projections, V inline after
            h0, dma0, trans0, kq0, v0_th = ltp_thunks(0, x_first_tile=x_first)
            q0, k0, v0 = h0
            emit_ltp(dma0, trans0, kq0)
            for th in v0_th:
                th()
            wps0 = phase_scores(0, q0, k0)

            # batch 1 load/KQ with batch 0's w-phase woven in
            h1, dma1, trans1, kq1, v1_th = ltp_thunks(1)
            q1, k1, v1 = h1
            emit_ltp(dma1, trans1, kq1, extra=final_thunks(0, wps0, v0))
            wps1 = phase_scores(1, q1, k1)

            # tail: batch 1's V projections woven through its w-phase
            fin1 = final_thunks(1, wps1, v1)
            vq = list(v1_th)
            for th in vq[:4]:
                th()
            vi = 4
            for i, th in enumerate(fin1):
                th()
                if vi < len(vq) and i % 2 == 1:
                    vq[vi]()
                    vi += 1
            while vi < len(vq):
                vq[vi]()
                vi += 1

    nc.compile()
    return nc


_NC_CACHE = None


def _get_nc():
    global _NC_CACHE
    if _NC_CACHE is None:
        _NC_CACHE = build_nc()
    return _NC_CACHE


def make_in_maps(inputs, W_q, W_k, W_v):
    inputs = np.ascontiguousarray(np.asarray(inputs, dtype=np.float32))
    W_q = np.ascontiguousarray(np.asarray(W_q, dtype=np.float32))
    W_k = np.ascontiguousarray(np.asarray(W_k, dtype=np.float32))
    W_v = np.ascontiguousarray(np.asarray(W_v, dtype=np.float32))
    return [
        {
            "inputs": inputs[i * B_PER_CORE : (i + 1) * B_PER_CORE],
            "W_q": W_q,
            "W_k": W_k,
            "W_v": W_v,
        }
        for i in range(N_CORES)
    ]


def kernel(**inputs) -> np.ndarray:
    nc = _get_nc()
    in_maps = make_in_maps(
        inputs["inputs"], inputs["W_q"], inputs["W_k"], inputs["W_v"]
    )
    res = run_bass_kernel_spmd(nc, in_maps, core_ids=list(range(N_CORES)))
    return np.concatenate(
        [res.results[i]["out"] for i in range(N_CORES)], axis=0
    ).astype(np.float32)


# revision 20
# speedup vs baseline: 1.2357x; 1.2357x over previous
"""Trainium2 Bass kernel for batched single-head attention with seq-sum pooling.

Reference computation (B=16, S=2048, D=512, fp32):
    q = x @ W_q ; k = x @ W_k ; v = x @ W_v          per batch  [S, D]
    scores = q @ k.T / sqrt(D)                        [S, S]
    attn = softmax(scores, axis=-1)
    out_b = sum_s (attn @ v)[s, :]                    [D]

Key algebraic restructure: the final sum over query positions commutes with
the attn @ v matmul:
    out_b = (sum_q attn[q, :]) @ v = (r^T E) @ v
where E = exp(scores / sqrt(D)) and r[q] = 1 / rowsum_q(E).  This removes the
second [S,S]x[S,D] matmul (~36% of the FLOPs) and replaces it with one
[1,S]x[S,S] column-sum matmul plus one [1,S]x[S,D] matvec.

Sharding: pure data parallelism over batch — 2 batch elements per core on 8
NeuronCores, weights replicated, no collectives.  Host concatenates per-core
[2, D] outputs.

Matmuls run as float32r (fp32 data, relaxed-precision PE mode, 4x fp32
throughput; measured rel error ~1.5e-4).  Emission is software-pipelined so
the PE sees a dense matmul stream: X-transposes run one s-chunk ahead woven
between projection groups, batch 0's w-phase weaves into batch 1's
projections, and batch 1's V projections are deferred to weave through the
tail w-phase — PE-idle windows >3.4us would drop the HAM clock gate from
2.4GHz to 1.2GHz (transpose-mode matmuls do not count as PE activity).
"""

import sys

sys.path.insert(0, "/opt/trn_rl_repo")

import numpy as np

import concourse.bass as bass
import concourse.mybir as mybir
import concourse.tile as tile
from concourse import bacc
from concourse.bass_utils import run_bass_kernel_spmd
from concourse.masks import make_identity

B, S, D = 16, 2048, 512
P = 128
N_CORES = 8
B_PER_CORE = B // N_CORES  # 2
SCALE = 1.0 / float(np.sqrt(D))

F32 = mybir.dt.float32
F32R = mybir.dt.float32r

N_ST = S // P  # 16 s-tiles (partition tiles of the sequence dim)
N_DT = D // P  # 4 d-tiles (partition tiles of the feature dim)
NCH = 512  # moving free dim per matmul (one PSUM bank of fp32)
N_SC = S // NCH  # 4 s-chunks of the sequence dim
N_KC = S // NCH  # 4 k-chunks of the key dim


def build_nc():
    nc = bacc.Bacc("TRN2", target_bir_lowering=False, debug=False, num_devices=N_CORES)
    x_ext = nc.dram_tensor(
        "inputs", [B_PER_CORE, S, D], F32, kind="ExternalInput"
    ).ap()
    wq_ext = nc.dram_tensor("W_q", [D, D], F32, kind="ExternalInput").ap()
    wk_ext = nc.dram_tensor("W_k", [D, D], F32, kind="ExternalInput").ap()
    wv_ext = nc.dram_tensor("W_v", [D, D], F32, kind="ExternalInput").ap()
    out_ext = nc.dram_tensor("out", [B_PER_CORE, D], F32, kind="ExternalOutput").ap()

    with tile.TileContext(nc) as tc:
        with (
            tc.tile_pool(name="const", bufs=1) as const_pool,
            tc.tile_pool(name="w", bufs=1) as w_pool,
            tc.tile_pool(name="xin", bufs=3) as xin_pool,
            tc.tile_pool(name="xt", bufs=1) as xt_pool,
            tc.tile_pool(name="qkv", bufs=1) as qkv_pool,
            tc.tile_pool(name="e", bufs=2) as e_pool,
            tc.tile_pool(name="soft", bufs=4) as soft_pool,
            tc.tile_pool(name="wvec", bufs=1) as wvec_pool,
            tc.tile_pool(name="scps", bufs=2, space="PSUM") as sc_psum,
            tc.tile_pool(name="gpps", bufs=2, space="PSUM") as gp_psum,
            tc.tile_pool(name="wps", bufs=1, space="PSUM") as w_psum,
        ):
            ident_f = const_pool.tile([P, P], F32)
            make_identity(nc, ident_f[:])
            ident = const_pool.tile([P, P], F32R)
            nc.vector.tensor_copy(ident[:], ident_f[:])
            one_f = const_pool.tile([1, 2], F32)
            nc.gpsimd.memset(one_f[:], 1.0)
            one_t = const_pool.tile([1, 2], F32R)
            nc.vector.tensor_copy(one_t[:], one_f[:])

            # Interleave batch 0's first x chunks with the weight loads on the
            # SWDGE (cast) queue so each lands just before the PE needs it:
            # x0 gates the first transposes, W_k the first projection group.
            def dma_x_chunk(b, sc):
                x_tile = xin_pool.tile([P, 4, D], F32R, tag="xin")
                nc.gpsimd.dma_start(
                    out=x_tile[:],
                    in_=x_ext[b, sc * NCH : (sc + 1) * NCH, :].rearrange(
                        "(t p) d -> p t d", p=P
                    ),
                )
                return x_tile

            w_tiles = {}

            def dma_w(name, ext):
                w_s = w_pool.tile([P, N_DT, D], F32R, tag=name)
                nc.gpsimd.dma_start(
                    out=w_s[:], in_=ext.rearrange("(t p) e -> p t e", p=P)
                )
                w_tiles[name] = w_s

            x_prefetch = [dma_x_chunk(0, 0)]
            dma_w("wk", wk_ext)
            x_prefetch.append(dma_x_chunk(0, 1))
            dma_w("wq", wq_ext)
            x_prefetch.append(dma_x_chunk(0, 2))
            dma_w("wv", wv_ext)
            wk_s, wq_s, wv_s = w_tiles["wk"], w_tiles["wq"], w_tiles["wv"]

            # ---------- thunk builders (emission deferred for interleaving) --

            def ltp_thunks(b, prefetched=()):
                """Load + transpose + QT/KT projection thunks for batch b.

                Returns (handles, dma_thunks[sc], trans_unit_thunks[sc] (4
                each: one s-tile = 4 PE transposes into one PSUM bank + one
                wide DVE copy), kq_group_thunks[sc] (8 each), v (16))."""
                xt_s = xt_pool.tile([P, N_DT, S], F32R, tag="xt")
                qt_s = qkv_pool.tile([P, N_DT, S], F32R, tag="qt")
                kt_s = qkv_pool.tile([P, N_DT, S], F32R, tag="kt")
                v_s = qkv_pool.tile([P, N_ST, D], F32R, tag="v")
                x_tiles = [None] * N_SC
                for i, xt_tile in enumerate(prefetched):
                    x_tiles[i] = xt_tile

                def make_dma(sc):
                    def th():
                        x_tiles[sc] = dma_x_chunk(b, sc)

                    return th

                def make_trans_unit(sc, t_i):
                    def th():
                        st = sc * 4 + t_i
                        tp = gp_psum.tile([P, N_DT * P], F32R, tag="gp")
                        for dt_i in range(N_DT):
                            nc.tensor.matmul(
                                tp[:, dt_i * P : (dt_i + 1) * P],
                                x_tiles[sc][:, t_i, dt_i * P : (dt_i + 1) * P],
                                ident[:],
                                is_transpose=True,
                                start=True,
                                stop=True,
                                skip_group_check=True,
                            )
                        nc.vector.tensor_copy(
                            xt_s[:, :, st * P : (st + 1) * P],
                            tp[:].rearrange("p (t c) -> p t c", t=N_DT),
                        )

                    return th

                def make_kq(sc, w_src, dst, et):
                    def th():
                        mp = gp_psum.tile([P, NCH], F32, tag="gp")
                        for kd in range(N_DT):
                            nc.tensor.matmul(
                                mp[:],
                                w_src[:, kd, et * P : (et + 1) * P],
                                xt_s[:, kd, sc * NCH : (sc + 1) * NCH],
                                start=(kd == 0),
                                stop=(kd == N_DT - 1),
                            )
                        nc.vector.tensor_copy(
                            dst[:, et, sc * NCH : (sc + 1) * NCH], mp[:]
                        )

                    return th

                def make_v(st):
                    def th():
                        mp = gp_psum.tile([P, NCH], F32, tag="gp")
                        for kd in range(N_DT):
                            nc.tensor.matmul(
                                mp[:],
                                xt_s[:, kd, st * P : (st + 1) * P],
                                wv_s[:, kd, :],
                                start=(kd == 0),
                                stop=(kd == N_DT - 1),
                            )
                        nc.vector.tensor_copy(v_s[:, st, :], mp[:])

                    return th

                dma_th = [None if x_tiles[sc] is not None else make_dma(sc)
                          for sc in range(N_SC)]
                trans_th = [
                    [make_trans_unit(sc, t_i) for t_i in range(4)]
                    for sc in range(N_SC)
                ]
                kq_th = [
                    [make_kq(sc, w_src, dst, et)
                     for w_src, dst in ((wk_s, kt_s), (wq_s, qt_s))
                     for et in range(N_DT)]
                    for sc in range(N_SC)
                ]
                v_th = [make_v(st) for st in range(N_ST)]
                return (qt_s, kt_s, v_s), dma_th, trans_th, kq_th, v_th

            def emit_ltp(dma_th, trans_th, kq_th, extra=None):
                """Emit load/transpose/KQ-projection stream.  Transposes for
                chunk sc+1 and `extra` thunks weave between chunk sc's
                projection groups so the PE stream stays dense."""
                extra = list(extra) if extra else []
                ei = 0
                if dma_th[0] is not None:
                    dma_th[0]()
                    dma_th[0] = None
                for th in trans_th[0]:
                    th()
                for sc in range(N_SC):
                    for j in (sc + 1, sc + 2):
                        if j < N_SC and dma_th[j] is not None:
                            dma_th[j]()
                            dma_th[j] = None
                    nxt = trans_th[sc + 1] if sc + 1 < N_SC else []
                    groups = kq_th[sc]
                    ti = 0
                    for g_i, g in enumerate(groups):
                        g()
                        while ti < len(nxt) and ti * len(groups) < (g_i + 1) * len(nxt):
                            nxt[ti]()
                            ti += 1
                        if ei < len(extra):
                            extra[ei]()
                            ei += 1
                    while ti < len(nxt):
                        nxt[ti]()
                        ti += 1
                while ei < len(extra):
                    extra[ei]()
                    ei += 1

            def emit_scores_qt(qt_s, kt_s, qt):
                """scores + exp + rowsum + reciprocal for one q-tile."""
                e_t = e_pool.tile([P, S], F32R, tag="e")
                rsum = soft_pool.tile([P, N_KC], F32, tag="rsum")
                for kc in range(N_KC):
                    sp = sc_psum.tile([P, NCH], F32, tag="sc")
                    for et in range(N_DT):
                        nc.tensor.matmul(
                            sp[:],
                            qt_s[:, et, qt * P : (qt + 1) * P],
                            kt_s[:, et, kc * NCH : (kc + 1) * NCH],
                            start=(et == 0),
                            stop=(et == N_DT - 1),
                        )
                    nc.scalar.activation(
                        e_t[:, kc * NCH : (kc + 1) * NCH],
                        sp[:],
                        mybir.ActivationFunctionType.Exp,
                        scale=SCALE,
                        accum_out=rsum[:, kc : kc + 1],
                    )
                rtot = soft_pool.tile([P, 1], F32, tag="rtot")
                nc.vector.reduce_sum(rtot[:], rsum[:], axis=mybir.AxisListType.X)
                rrec = soft_pool.tile([P, 1], F32, tag="rrec")
                nc.vector.reciprocal(rrec[:], rtot[:])
                # f32r matmuls need a full 128-wide stationary operand; broadcast
                # r across all columns so every PSUM output row equals r^T E.
                r_t = soft_pool.tile([P, P], F32R, tag="r")
                nc.vector.tensor_copy(r_t[:], rrec[:, 0:1].broadcast_to([P, P]))
                return e_t, r_t

            def emit_colsum_qt(w_ps, e_t, r_t, qt):
                """w_ps[:, kc, :] += bcast(r_qt)^T @ E_qt (every row = colsum)."""
                for kc in range(N_KC):
                    nc.tensor.matmul(
                        w_ps[:, kc, :],
                        r_t[:],
                        e_t[:, kc * NCH : (kc + 1) * NCH],
                        start=(qt == 0),
                        stop=(qt == N_ST - 1),
                        skip_group_check=True,
                    )

            def phase_scores(b, qt_s, kt_s):
                w_ps = w_psum.tile([P, N_KC, NCH], F32, tag="w")
                prev = None
                for qt in range(N_ST):
                    cur = emit_scores_qt(qt_s, kt_s, qt)
                    if prev is not None:
                        emit_colsum_qt(w_ps, prev[0], prev[1], qt - 1)
                    prev = cur
                emit_colsum_qt(w_ps, prev[0], prev[1], N_ST - 1)
                return w_ps

            def final_thunks(b, w_ps, v_s):
                """w-phase thunks: 4 ACT copies of w, then 16 (PE transpose +
                DVE broadcast), then 16 final accumulation matmuls, then the
                output copy + DMA.  Emitted interleaved by the caller."""
                w_sb = wvec_pool.tile([1, S], F32R, tag="wsb")
                o_ps = sc_psum.tile([P, NCH], F32, tag="sc")
                wt_pads = {}
                thunks = []

                def make_wcopy(kc):
                    def th():
                        nc.scalar.copy(
                            w_sb[:, kc * NCH : (kc + 1) * NCH], w_ps[0:1, kc, :]
                        )

                    return th

                def make_wtrans(kt):
                    def th():
                        tp = gp_psum.tile([P, 2], F32, tag="gp")
                        nc.tensor.matmul(
                            tp[:],
                            w_sb[0:1, kt * P : (kt + 1) * P],
                            one_t[0:1, 0:2],
                            start=True,
                            stop=True,
                        )
                        wt_pad = wvec_pool.tile([P, P], F32R, tag=f"wtp{kt % 4}")
                        nc.vector.tensor_copy(
                            wt_pad[:], tp[:, 0:1].broadcast_to([P, P])
                        )
                        wt_pads[kt] = wt_pad

                    return th

                def make_final(st):
                    def th():
                        nc.tensor.matmul(
                            o_ps[:],
                            wt_pads[st][:],
                            v_s[:, st, :],
                            start=(st == 0),
                            stop=(st == N_ST - 1),
                            skip_group_check=True,
                        )

                    return th

                def make_out():
                    def th():
                        o_sb = wvec_pool.tile([1, NCH], F32, tag="osb")
                        nc.scalar.copy(o_sb[:], o_ps[0:1, :])
                        nc.sync.dma_start(out=out_ext[b : b + 1, :], in_=o_sb[:])

                    return th

                for kc in range(N_KC):
                    thunks.append(make_wcopy(kc))
                for kt in range(N_ST):
                    thunks.append(make_wtrans(kt))
                    if kt >= 3:
                        thunks.append(make_final(kt - 3))
                for st in range(N_ST - 3, N_ST):
                    thunks.append(make_final(st))
                thunks.append(make_out())
                return thunks

            # ------------------------- emission ------------------------------

            # batch 0: transposes woven into KQ projections, V inline after
            h0, dma0, trans0, kq0, v0_th = ltp_thunks(0, prefetched=x_prefetch)
            q0, k0, v0 = h0
            emit_ltp(dma0, trans0, kq0)
            for th in v0_th:
                th()
            wps0 = phase_scores(0, q0, k0)

            # batch 1 load/KQ with batch 0's w-phase woven in
            h1, dma1, trans1, kq1, v1_th = ltp_thunks(1)
            q1, k1, v1 = h1
            emit_ltp(dma1, trans1, kq1, extra=final_thunks(0, wps0, v0))
            wps1 = phase_scores(1, q1, k1)

            # tail: batch 1's V projections woven through its w-phase
            fin1 = final_thunks(1, wps1, v1)
            vq = list(v1_th)
            for th in vq[:4]:
                th()
            vi = 4
            for i, th in enumerate(fin1):
                th()
                if vi < len(vq) and i % 2 == 1:
                    vq[vi]()
                    vi += 1
            while vi < len(vq):
                vq[vi]()
                vi += 1

    nc.compile()
    return nc


_NC_CACHE = None


def _get_nc():
    global _NC_CACHE
    if _NC_CACHE is None:
        _NC_CACHE = build_nc()
    return _NC_CACHE


def make_in_maps(inputs, W_q, W_k, W_v):
    inputs = np.ascontiguousarray(np.asarray(inputs, dtype=np.float32))
    W_q = np.ascontiguousarray(np.asarray(W_q, dtype=np.float32))
    W_k = np.ascontiguousarray(np.asarray(W_k, dtype=np.float32))
    W_v = np.ascontiguousarray(np.asarray(W_v, dtype=np.float32))
    return [
        {
            "inputs": inputs[i * B_PER_CORE : (i + 1) * B_PER_CORE],
            "W_q": W_q,
            "W_k": W_k,
            "W_v": W_v,
        }
        for i in range(N_CORES)
    ]


def kernel(**inputs) -> np.ndarray:
    nc = _get_nc()
    in_maps = make_in_maps(
        inputs["inputs"], inputs["W_q"], inputs["W_k"], inputs["W_v"]
    )
    res = run_bass_kernel_spmd(nc, in_maps, core_ids=list(range(N_CORES)))
    return np.concatenate(
        [res.results[i]["out"] for i in range(N_CORES)], axis=0
    ).astype(np.float32)


# revision 21
# speedup vs baseline: 1.2636x; 1.0226x over previous
"""Trainium2 Bass kernel for batched single-head attention with seq-sum pooling.

Reference computation (B=16, S=2048, D=512, fp32):
    q = x @ W_q ; k = x @ W_k ; v = x @ W_v          per batch  [S, D]
    scores = q @ k.T / sqrt(D)                        [S, S]
    attn = softmax(scores, axis=-1)
    out_b = sum_s (attn @ v)[s, :]                    [D]

Key algebraic restructure: the final sum over query positions commutes with
the attn @ v matmul:
    out_b = (sum_q attn[q, :]) @ v = (r^T E) @ v
where E = exp(scores / sqrt(D)) and r[q] = 1 / rowsum_q(E).  This removes the
second [S,S]x[S,D] matmul (~36% of the FLOPs) and replaces it with one
[1,S]x[S,S] column-sum matmul plus one [1,S]x[S,D] matvec.

Sharding: pure data parallelism over batch — 2 batch elements per core on 8
NeuronCores, weights replicated, no collectives.  Host concatenates per-core
[2, D] outputs.

Matmul operands are bf16 (fp32 PSUM accumulation), which streams the PE at
~217 ns per [128x128]x[128x512] matmul and allows the X transpose to ride the
DMA crossbar (f32 -> bf16 cast-DMA to a DRAM scratch, then hardware
transpose-DMA into SBUF) instead of burning TensorE cycles — transpose-mode
matmuls also don't count as PE activity for the HAM clock gate and would
re-throttle the array to 1.2 GHz.  Measured end-to-end rel error ~2e-3
(reference tolerance 2e-2).  Emission is software-pipelined: batch 0's
w-phase weaves into batch 1's projections and batch 1's V projections weave
through the tail w-phase so the PE never idles long enough to go cold.
"""

import sys

sys.path.insert(0, "/opt/trn_rl_repo")

import numpy as np

import concourse.bass as bass
import concourse.mybir as mybir
import concourse.tile as tile
from concourse import bacc
from concourse.bass_utils import run_bass_kernel_spmd

B, S, D = 16, 2048, 512
P = 128
N_CORES = 8
B_PER_CORE = B // N_CORES  # 2
SCALE = 1.0 / float(np.sqrt(D))

F32 = mybir.dt.float32
BF16 = mybir.dt.bfloat16

N_ST = S // P  # 16 s-tiles (partition tiles of the sequence dim)
N_DT = D // P  # 4 d-tiles (partition tiles of the feature dim)
NCH = 512  # moving free dim per matmul (one fp32 PSUM bank)
N_SC = S // NCH  # 4 s-chunks of the sequence dim
N_KC = S // NCH  # 4 k-chunks of the key dim


def build_nc():
    nc = bacc.Bacc("TRN2", target_bir_lowering=False, debug=False, num_devices=N_CORES)
    x_ext = nc.dram_tensor(
        "inputs", [B_PER_CORE, S, D], F32, kind="ExternalInput"
    ).ap()
    wq_ext = nc.dram_tensor("W_q", [D, D], F32, kind="ExternalInput").ap()
    wk_ext = nc.dram_tensor("W_k", [D, D], F32, kind="ExternalInput").ap()
    wv_ext = nc.dram_tensor("W_v", [D, D], F32, kind="ExternalInput").ap()
    out_ext = nc.dram_tensor("out", [B_PER_CORE, D], F32, kind="ExternalOutput").ap()
    # DRAM scratch for the bf16 copy of x (input to the transpose-DMA)
    xb_dram = nc.dram_tensor("xb_scratch", [B_PER_CORE, S, D], BF16).ap()

    with tile.TileContext(nc) as tc:
        with (
            tc.tile_pool(name="const", bufs=1) as const_pool,
            tc.tile_pool(name="w", bufs=1) as w_pool,
            tc.tile_pool(name="xt", bufs=2) as xt_pool,
            tc.tile_pool(name="qkv", bufs=2) as qkv_pool,
            tc.tile_pool(name="e", bufs=3) as e_pool,
            tc.tile_pool(name="soft", bufs=4) as soft_pool,
            tc.tile_pool(name="wvec", bufs=2) as wvec_pool,
            tc.tile_pool(name="scps", bufs=2, space="PSUM") as sc_psum,
            tc.tile_pool(name="gpps", bufs=2, space="PSUM") as gp_psum,
            tc.tile_pool(name="wps", bufs=1, space="PSUM") as w_psum,
        ):
            one_t = const_pool.tile([1, 1], BF16)
            nc.gpsimd.memset(one_t[:], 1.0)

            # x cast chunks (SWDGE f32->bf16, DRAM->DRAM) and transpose-DMAs
            # (HWDGE xbar, DRAM->SBUF).  Pure DMA work — no PE involvement.
            def dma_cast_chunk(b, sc):
                nc.gpsimd.dma_start(
                    out=xb_dram[b, sc * NCH : (sc + 1) * NCH, :],
                    in_=x_ext[b, sc * NCH : (sc + 1) * NCH, :],
                )

            def dma_transpose_chunk(b, sc, xt_s):
                for dt_i in range(N_DT):
                    nc.sync.dma_start(
                        out=xt_s[:, dt_i, sc * NCH : (sc + 1) * NCH],
                        in_=xb_dram[
                            b, sc * NCH : (sc + 1) * NCH, dt_i * P : (dt_i + 1) * P
                        ],
                        transpose=True,
                    )

            w_tiles = {}

            def dma_w(name, ext):
                w_s = w_pool.tile([P, N_DT, D], BF16, tag=name)
                nc.gpsimd.dma_start(
                    out=w_s[:], in_=ext.rearrange("(t p) e -> p t e", p=P)
                )
                w_tiles[name] = w_s

            # Interleave batch 0's cast chunks with the weight loads on the
            # SWDGE queue so each lands just before the PE needs it.
            xt0_s = xt_pool.tile([P, N_DT, S], BF16, tag="xt")
            dma_cast_chunk(0, 0)
            dma_transpose_chunk(0, 0, xt0_s)
            dma_w("wk", wk_ext)
            dma_cast_chunk(0, 1)
            dma_transpose_chunk(0, 1, xt0_s)
            dma_w("wq", wq_ext)
            dma_cast_chunk(0, 2)
            dma_transpose_chunk(0, 2, xt0_s)
            dma_w("wv", wv_ext)
            dma_cast_chunk(0, 3)
            dma_transpose_chunk(0, 3, xt0_s)
            wk_s, wq_s, wv_s = w_tiles["wk"], w_tiles["wq"], w_tiles["wv"]

            # ---------- thunk builders (emission deferred for interleaving) --

            def proj_thunks(b, xt_s):
                """QT/KT/V projection thunks for batch b (xt_s already being
                filled by the transpose-DMAs)."""
                qt_s = qkv_pool.tile([P, N_DT, S], BF16, tag="qt")
                kt_s = qkv_pool.tile([P, N_DT, S], BF16, tag="kt")
                v_s = qkv_pool.tile([P, N_ST, D], BF16, tag="v")

                def make_kq(sc, w_src, dst, et):
                    def th():
                        mp = gp_psum.tile([P, NCH], F32, tag="gp")
                        for kd in range(N_DT):
                            nc.tensor.matmul(
                                mp[:],
                                w_src[:, kd, et * P : (et + 1) * P],
                                xt_s[:, kd, sc * NCH : (sc + 1) * NCH],
                                start=(kd == 0),
                                stop=(kd == N_DT - 1),
                            )
                        nc.vector.tensor_copy(
                            dst[:, et, sc * NCH : (sc + 1) * NCH], mp[:]
                        )

                    return th

                def make_v(st):
                    def th():
                        mp = gp_psum.tile([P, NCH], F32, tag="gp")
                        for kd in range(N_DT):
                            nc.tensor.matmul(
                                mp[:],
                                xt_s[:, kd, st * P : (st + 1) * P],
                                wv_s[:, kd, :],
                                start=(kd == 0),
                                stop=(kd == N_DT - 1),
                            )
                        nc.vector.tensor_copy(v_s[:, st, :], mp[:])

                    return th

                kq_th = [
                    make_kq(sc, w_src, dst, et)
                    for sc in range(N_SC)
                    for w_src, dst in ((wk_s, kt_s), (wq_s, qt_s))
                    for et in range(N_DT)
                ]
                v_th = [make_v(st) for st in range(N_ST)]
                return (qt_s, kt_s, v_s), kq_th, v_th

            def emit_interleaved(main_th, extra_th):
                """Emit main thunks with extra thunks woven evenly between."""
                extra_th = list(extra_th)
                ei = 0
                for i, th in enumerate(main_th):
                    th()
                    while ei < len(extra_th) and ei * len(main_th) <= (i + 1) * len(
                        extra_th
                    ):
                        extra_th[ei]()
                        ei += 1
                while ei < len(extra_th):
                    extra_th[ei]()
                    ei += 1

            def emit_scores_qt(qt_s, kt_s, qt):
                """scores + exp + rowsum + reciprocal for one q-tile."""
                e_t = e_pool.tile([P, S], BF16, tag="e")
                rsum = soft_pool.tile([P, N_KC], F32, tag="rsum")
                for kc in range(N_KC):
                    sp = sc_psum.tile([P, NCH], F32, tag="sc")
                    for et in range(N_DT):
                        nc.tensor.matmul(
                            sp[:],
                            qt_s[:, et, qt * P : (qt + 1) * P],
                            kt_s[:, et, kc * NCH : (kc + 1) * NCH],
                            start=(et == 0),
                            stop=(et == N_DT - 1),
                        )
                    nc.scalar.activation(
                        e_t[:, kc * NCH : (kc + 1) * NCH],
                        sp[:],
                        mybir.ActivationFunctionType.Exp,
                        scale=SCALE,
                        accum_out=rsum[:, kc : kc + 1],
                    )
                rtot = soft_pool.tile([P, 1], F32, tag="rtot")
                nc.vector.reduce_sum(rtot[:], rsum[:], axis=mybir.AxisListType.X)
                rrec = soft_pool.tile([P, 1], F32, tag="rrec")
                nc.vector.reciprocal(rrec[:], rtot[:])
                r_t = soft_pool.tile([P, 1], BF16, tag="r")
                nc.vector.tensor_copy(r_t[:], rrec[:])
                return e_t, r_t

            def emit_colsum_qt(w_ps, e_t, r_t, qt):
                """w_ps[0, kc, :] += r_qt^T @ E_qt (column sums of attn)."""
                for kc in range(N_KC):
                    nc.tensor.matmul(
                        w_ps[0:1, kc, :],
                        r_t[:, 0:1],
                        e_t[:, kc * NCH : (kc + 1) * NCH],
                        start=(qt == 0),
                        stop=(qt == N_ST - 1),
                        skip_group_check=True,
                    )

            def phase_scores(b, qt_s, kt_s):
                w_ps = w_psum.tile([1, N_KC, NCH], F32, tag="w")
                prev = None
                for qt in range(N_ST):
                    cur = emit_scores_qt(qt_s, kt_s, qt)
                    if prev is not None:
                        emit_colsum_qt(w_ps, prev[0], prev[1], qt - 1)
                    prev = cur
                emit_colsum_qt(w_ps, prev[0], prev[1], N_ST - 1)
                return w_ps

            def final_thunks(b, w_ps, v_s):
                """w-phase thunks: 4 ACT copies of w, then 16 (PE transpose +
                DVE column copy), then 16 final accumulation matmuls, then the
                output copy + DMA.  Emitted interleaved by the caller."""
                w_sb = wvec_pool.tile([1, S], BF16, tag="wsb")
                wt_sb = wvec_pool.tile([P, N_ST], BF16, tag="wtsb")
                o_ps = sc_psum.tile([P, NCH], F32, tag="sc")
                thunks = []

                def make_wcopy(kc):
                    def th():
                        nc.scalar.copy(
                            w_sb[:, kc * NCH : (kc + 1) * NCH], w_ps[0:1, kc, :]
                        )

                    return th

                def make_wtrans(kt):
                    def th():
                        tp = gp_psum.tile([P, 1], F32, tag="gp")
                        nc.tensor.matmul(
                            tp[:],
                            w_sb[0:1, kt * P : (kt + 1) * P],
                            one_t[0:1, 0:1],
                            start=True,
                            stop=True,
                        )
                        nc.vector.tensor_copy(wt_sb[:, kt : kt + 1], tp[:])

                    return th

                def make_final(st):
                    def th():
                        nc.tensor.matmul(
                            o_ps[0:1, :],
                            wt_sb[:, st : st + 1],
                            v_s[:, st, :],
                            start=(st == 0),
                            stop=(st == N_ST - 1),
                            skip_group_check=True,
                        )

                    return th

                def out_th():
                    o_sb = wvec_pool.tile([1, NCH], F32, tag="osb")
                    nc.scalar.copy(o_sb[:], o_ps[0:1, :])
                    nc.sync.dma_start(out=out_ext[b : b + 1, :], in_=o_sb[:])

                for kc in range(N_KC):
                    thunks.append(make_wcopy(kc))
                for kt in range(N_ST):
                    thunks.append(make_wtrans(kt))
                    if kt >= 3:
                        thunks.append(make_final(kt - 3))
                for st in range(N_ST - 3, N_ST):
                    thunks.append(make_final(st))
                thunks.append(out_th)
                return thunks

            # ------------------------- emission ------------------------------

            # batch 0 projections (V inline — final(0) reads it)
            h0, kq0, v0_th = proj_thunks(0, xt0_s)
            q0, k0, v0 = h0
            emit_interleaved(kq0 + v0_th, [])

            # batch 1 x-DMAs: pure DMA, runs during batch 0's scores
            xt1_s = xt_pool.tile([P, N_DT, S], BF16, tag="xt")
            for sc in range(N_SC):
                dma_cast_chunk(1, sc)
                dma_transpose_chunk(1, sc, xt1_s)

            wps0 = phase_scores(0, q0, k0)

            # batch 1 projections with batch 0's w-phase woven in
            h1, kq1, v1_th = proj_thunks(1, xt1_s)
            q1, k1, v1 = h1
            emit_interleaved(kq1, final_thunks(0, wps0, v0))

            wps1 = phase_scores(1, q1, k1)

            # tail: batch 1's V projections woven through its w-phase
            fin1 = final_thunks(1, wps1, v1)
            for th in v1_th[:4]:
                th()
            vi = 4
            for i, th in enumerate(fin1):
                th()
                if vi < len(v1_th) and i % 2 == 1:
                    v1_th[vi]()
                    vi += 1
            while vi < len(v1_th):
                v1_th[vi]()
                vi += 1

    nc.compile()
    return nc


_NC_CACHE = None


def _get_nc():
    global _NC_CACHE
    if _NC_CACHE is None:
        _NC_CACHE = build_nc()
    return _NC_CACHE


def make_in_maps(inputs, W_q, W_k, W_v):
    inputs = np.ascontiguousarray(np.asarray(inputs, dtype=np.float32))
    W_q = np.ascontiguousarray(np.asarray(W_q, dtype=np.float32))
    W_k = np.ascontiguousarray(np.asarray(W_k, dtype=np.float32))
    W_v = np.ascontiguousarray(np.asarray(W_v, dtype=np.float32))
    return [
        {
            "inputs": inputs[i * B_PER_CORE : (i + 1) * B_PER_CORE],
            "W_q": W_q,
            "W_k": W_k,
            "W_v": W_v,
        }
        for i in range(N_CORES)
    ]


def kernel(**inputs) -> np.ndarray:
    nc = _get_nc()
    in_maps = make_in_maps(
        inputs["inputs"], inputs["W_q"], inputs["W_k"], inputs["W_v"]
    )
    res = run_bass_kernel_spmd(nc, in_maps, core_ids=list(range(N_CORES)))
    return np.concatenate(
        [res.results[i]["out"] for i in range(N_CORES)], axis=0
    ).astype(np.float32)
